# revision 5
# baseline (speedup 1.0000x reference)
"""Dense supervised-contrastive loss on 8 Trainium2 NeuronCores.

Strategy (matches the sharding hint):
  - image-per-core: core k DMAs image k's feature map (256, 9216) to SBUF and
    extracts its sampled anchor columns with gpsimd.ap_gather.
  - AllGather the (256, M_MAX)-padded anchor slabs -> every core holds the
    full (256, 8192) anchor matrix; columns are grouped by image (= by
    producing core), each t contributing a contiguous 256-column block.
  - row-shard the (8192 x 8192) similarity matrix: core k owns rows
    [1024k, 1024k+1024) (8 row-blocks of 128, each inside a single t).
  - per row-block: PE computes the Gram slice (fp32, K=256 via 2 accumulating
    matmuls), ACT does exp with accum_out (row sums), gpsimd packs the
    equal-label column windows, ACT Ln passes give the SupCon log-prob sums.

Math (shift-invariant form of the reference; exact in exact arithmetic):
  E_ij = exp(2 * x_i . x_j)  with x normalized (the 2 = sqrt(2)^2 folds the
  temperature 0.5 into the normalization scale).
  ns_i  = sum_j E_ij - sum_{j in eq(i)} E_ij              (negatives sum)
  PosSum_i = sum_{j in eq(i)} (dot_ij - log(E_ij + ns_i)) - (2 - log(e^2+ns_i))
  contrib_i = (PosSum_i - ... ) * 1/n_pos ;  loss = -mean_i contrib_i
The diagonal term uses dot_ii = 2.0 exactly (x normalized), error ~1e-6 abs.
"""

import os
import numpy as np

import concourse.bass as bass
import concourse.mybir as mybir
from concourse import bacc, tile

f32 = mybir.dt.float32
i16 = mybir.dt.int16

N_CORES = 8
N, C, H, W = 8, 256, 96, 96
HW = H * W
T, V = 32, 256
TV = T * V  # 8192
E2 = float(np.exp(2.0).astype(np.float64))  # e^2


def _wrap16(idx, P=128):
    """Wrap an index list into gpsimd layout: idx i at [i%16, i//16], replicated
    across the P//16 partition groups."""
    idx = np.asarray(idx, dtype=np.int16)
    L = len(idx)
    assert L % 16 == 0
    arr = np.zeros((16, L // 16), dtype=np.int16)
    arr[np.arange(L) % 16, np.arange(L) // 16] = idx
    return np.tile(arr, (P // 16, 1))


def _plan(batch_inds, sample_inds, labels):
    batch_inds = np.asarray(batch_inds).astype(np.int64)
    sample_inds = np.asarray(sample_inds).astype(np.int64)
    labels = np.asarray(labels).astype(np.int64)

    # image-grouped t order (stable within an image)
    t_perm = np.argsort(batch_inds, kind="stable")
    counts = np.bincount(batch_inds, minlength=N)          # t's per image
    M = counts * V                                         # columns per image
    M_MAX = int(M.max())
    img_colstart = np.concatenate([[0], np.cumsum(M)[:-1]])  # per image

    colstart_t = np.zeros(T, dtype=np.int64)               # per original t
    for pos, t in enumerate(t_perm):
        colstart_t[t] = pos * V

    # label multiplicities -> packed window size
    lab_sorted = labels[t_perm]
    mult = np.bincount(labels, minlength=19)
    C_MAX = int(mult[mult > 0].max())
    PACKW = C_MAX * V

    E_W = TV + 16   # E tile has 16 dummy columns of 1.0 at [TV, TV+16)

    per_core = []
    for k in range(N_CORES):
        # gather indices: this core's image's t's, in t_perm order
        own_ts = [t for t in t_perm if batch_inds[t] == k]
        gidx = np.concatenate([sample_inds[t] for t in own_ts]) if own_ts else np.zeros(0)
        gidx = np.concatenate([gidx, np.zeros(M_MAX - len(gidx))])
        gidx_w = _wrap16(gidx)

        # window pack indices per block (8 blocks; global block g = 8k+b)
        widx_blocks = []
        npad = np.zeros(8, dtype=np.float32)
        scl = np.zeros(8, dtype=np.float32)
        for b in range(8):
            g = 8 * k + b
            t_g = t_perm[g // 2]
            lab = labels[t_g]
            mates = [t for t in t_perm if labels[t] == lab]
            c_l = len(mates)
            w = np.concatenate(
                [np.arange(colstart_t[t], colstart_t[t] + V) for t in mates]
                + [np.full(PACKW - c_l * V, TV)]
            )
            widx_blocks.append(_wrap16(w))
            npad[b] = PACKW - c_l * V
            scl[b] = 1.0 / (c_l * V - 1)
        widx_w = np.concatenate(widx_blocks, axis=1)  # [128, 8*(PACKW//16)]

        per_core.append(
            dict(
                gidx=gidx_w,
                widx=widx_w,
                npad=np.tile(npad[None, :], (128, 1)).astype(np.float32),
                scale=np.tile(scl[None, :], (128, 1)).astype(np.float32),
            )
        )

    return dict(
        M=M.tolist(),
        M_MAX=M_MAX,
        img_colstart=img_colstart.tolist(),
        PACKW=PACKW,
        E_W=E_W,
        per_core=per_core,
    )


def _build(plan, debug=False):
    M_MAX = plan["M_MAX"]
    PACKW = plan["PACKW"]
    E_W = plan["E_W"]

    nc = bacc.Bacc(
        "TRN2",
        target_bir_lowering=False,
        debug=debug,
        num_devices=N_CORES,
    )

    feat_d = nc.dram_tensor("feat", [C, HW], f32, kind="ExternalInput")
    gidx_d = nc.dram_tensor("gidx", [128, M_MAX // 16], i16, kind="ExternalInput")
    widx_d = nc.dram_tensor("widx", [128, 8 * (PACKW // 16)], i16, kind="ExternalInput")
    npad_d = nc.dram_tensor("npad", [128, 8], f32, kind="ExternalInput")
    scale_d = nc.dram_tensor("scale", [128, 8], f32, kind="ExternalInput")
    out_d = nc.dram_tensor("contrib", [128, 8], f32, kind="ExternalOutput")

    with tile.TileContext(nc) as tc:
        with (
            tc.tile_pool(name="dram", bufs=1, space="DRAM") as dpool,
            tc.tile_pool(name="persist", bufs=1) as pp,
        ):
            xin = dpool.tile([C, M_MAX], f32)
            xg = dpool.tile([N_CORES * C, M_MAX], f32, addr_space="Shared")

            X0 = pp.tile([128, TV], f32)
            X1 = pp.tile([128, TV], f32)
            widx = pp.tile([128, 8 * (PACKW // 16)], i16)
            npad = pp.tile([128, 8], f32)
            scale = pp.tile([128, 8], f32)
            contrib = pp.tile([128, 8], f32)
            own0 = pp.tile([128, 1024], f32)
            own1 = pp.tile([128, 1024], f32)
            c_e2 = pp.tile([128, 1], f32)
            nc.vector.memset(c_e2[:], E2)

            nc.sync.dma_start(widx[:], widx_d[:])
            nc.sync.dma_start(npad[:], npad_d[:])
            nc.sync.dma_start(scale[:], scale_d[:])

            # ---- phase A: load own image, gather anchors, exchange ----
            with tc.tile_pool(name="gather", bufs=1) as gp:
                F0 = gp.tile([128, HW], f32)
                F1 = gp.tile([128, HW], f32)
                gidx = gp.tile([128, M_MAX // 16], i16)
                G0 = gp.tile([128, M_MAX], f32)
                G1 = gp.tile([128, M_MAX], f32)
                nc.sync.dma_start(F0[:], feat_d[0:128, :])
                nc.sync.dma_start(F1[:], feat_d[128:256, :])
                nc.sync.dma_start(gidx[:], gidx_d[:])
                nc.gpsimd.ap_gather(G0[:], F0[:], gidx[:], channels=128,
                                    num_elems=HW, d=1, num_idxs=M_MAX)
                nc.gpsimd.ap_gather(G1[:], F1[:], gidx[:], channels=128,
                                    num_elems=HW, d=1, num_idxs=M_MAX)
                nc.sync.dma_start(xin[0:128, :], G0[:])
                nc.sync.dma_start(xin[128:256, :], G1[:])

            nc.gpsimd.collective_compute(
                "AllGather",
                mybir.AluOpType.bypass,
                ins=[xin[:]],
                outs=[xg[:]],
                replica_groups=[list(range(N_CORES))],
            )

            # ---- phase B: reassemble full (256, 8192) X ----
            for r in range(N_CORES):
                Mr = plan["M"][r]
                if Mr == 0:
                    continue
                cs = plan["img_colstart"][r]
                nc.sync.dma_start(X0[:, cs:cs + Mr], xg[256 * r:256 * r + 128, 0:Mr])
                nc.sync.dma_start(X1[:, cs:cs + Mr], xg[256 * r + 128:256 * r + 256, 0:Mr])

            # ---- phase C: column-normalize X (fold sqrt(2) for temperature) ----
            with (
                tc.tile_pool(name="norm_sb", bufs=1) as np_sb,
                tc.tile_pool(name="sq", bufs=3) as sqp,
                tc.tile_pool(name="ss_ps", bufs=2, space="PSUM") as ssp,
            ):
                ones = np_sb.tile([128, 128], f32)
                s_t = np_sb.tile([128, TV], f32)
                r_t = np_sb.tile([128, TV], f32)
                nc.vector.memset(ones[:], 1.0)
                for cc in range(4):
                    c0 = cc * 2048
                    ss = ssp.tile([128, 2048], f32)
                    sqa = sqp.tile([128, 2048], f32, tag="sq")
                    nc.vector.tensor_mul(sqa[:], X0[:, c0:c0 + 2048], X0[:, c0:c0 + 2048])
                    for sub in range(4):
                        nc.tensor.matmul(ss[:, sub * 512:(sub + 1) * 512], ones[:],
                                         sqa[:, sub * 512:(sub + 1) * 512],
                                         start=True, stop=False)
                    sqb = sqp.tile([128, 2048], f32, tag="sq")
                    nc.vector.tensor_mul(sqb[:], X1[:, c0:c0 + 2048], X1[:, c0:c0 + 2048])
                    for sub in range(4):
                        nc.tensor.matmul(ss[:, sub * 512:(sub + 1) * 512], ones[:],
                                         sqb[:, sub * 512:(sub + 1) * 512],
                                         start=False, stop=True)
                    # s = sqrt(ss/2)  ->  1/s = sqrt(2)/||x||
                    nc.scalar.activation(s_t[:, c0:c0 + 2048], ss[:],
                                         mybir.ActivationFunctionType.Sqrt,
                                         scale=0.5)
                nc.vector.reciprocal(r_t[:], s_t[:])
                nc.vector.tensor_mul(X0[:], X0[:], r_t[:])
                nc.vector.tensor_mul(X1[:], X1[:], r_t[:])

            # ---- phase D: slice out this core's own row anchors ----
            pid = nc.gpsimd.partition_id()
            nc.gpsimd.dma_start(own0[:], X0[:, bass.ds(pid * 1024, 1024)])
            nc.gpsimd.dma_start(own1[:], X1[:, bass.ds(pid * 1024, 1024)])

            # ---- phase E: 8 row-blocks of the similarity/loss ----
            with (
                tc.tile_pool(name="g_ps", bufs=2, space="PSUM") as gps,
                tc.tile_pool(name="e_sb", bufs=2) as ep,
                tc.tile_pool(name="pack", bufs=2) as pk,
                tc.tile_pool(name="small", bufs=16) as sm,
            ):
                for b in range(8):
                    lt = own0[:, b * 128:(b + 1) * 128]
                    lt1 = own1[:, b * 128:(b + 1) * 128]
                    E_t = ep.tile([128, E_W], f32, tag="E")
                    sallp = sm.tile([128, 4], f32, tag="sallp")
                    for cc in range(4):
                        c0 = cc * 2048
                        g_ps = gps.tile([128, 2048], f32, tag="g")
                        for sub in range(4):
                            cs_ = sub * 512
                            nc.tensor.matmul(g_ps[:, cs_:cs_ + 512], lt,
                                             X0[:, c0 + cs_:c0 + cs_ + 512],
                                             start=True, stop=False)
                            nc.tensor.matmul(g_ps[:, cs_:cs_ + 512], lt1,
                                             X1[:, c0 + cs_:c0 + cs_ + 512],
                                             start=False, stop=True)
                        nc.scalar.activation(E_t[:, c0:c0 + 2048], g_ps[:],
                                             mybir.ActivationFunctionType.Exp,
                                             accum_out=sallp[:, cc:cc + 1])
                    nc.vector.memset(E_t[:, TV:E_W], 1.0)

                    packed = pk.tile([128, PACKW], f32, tag="pk")
                    nc.gpsimd.ap_gather(
                        packed[:], E_t[:],
                        widx[:, b * (PACKW // 16):(b + 1) * (PACKW // 16)],
                        channels=128, num_elems=E_W, d=1, num_idxs=PACKW)

                    s_all = sm.tile([128, 1], f32, tag="s_all")
                    s_eq = sm.tile([128, 1], f32, tag="s_eq")
                    ns = sm.tile([128, 1], f32, tag="ns")
                    lnE = sm.tile([128, 1], f32, tag="lnE")
                    lnEns = sm.tile([128, 1], f32, tag="lnEns")
                    l1 = sm.tile([128, 1], f32, tag="l1")
                    l2 = sm.tile([128, 1], f32, tag="l2")
                    u1 = sm.tile([128, 1], f32, tag="u1")
                    u2 = sm.tile([128, 1], f32, tag="u2")
                    scr = pk.tile([128, PACKW], f32, tag="scr")
                    scr2 = pk.tile([128, PACKW], f32, tag="scr2")

                    nc.vector.tensor_reduce(s_all[:], sallp[:], mybir.AxisListType.X,
                                            mybir.AluOpType.add)
                    nc.vector.tensor_reduce(s_eq[:], packed[:], mybir.AxisListType.X,
                                            mybir.AluOpType.add)
                    # ns = s_all - s_eq + npad
                    nc.vector.tensor_sub(u1[:], s_all[:], s_eq[:])
                    nc.vector.tensor_add(ns[:], u1[:], npad[:, b:b + 1])
                    # lnE = sum ln(E_pos);  lnEns = sum ln(E_pos + ns)
                    nc.scalar.activation(scr[:], packed[:],
                                         mybir.ActivationFunctionType.Ln,
                                         accum_out=lnE[:])
                    nc.scalar.activation(scr2[:], packed[:],
                                         mybir.ActivationFunctionType.Ln,
                                         bias=ns[:], accum_out=lnEns[:])
                    nc.scalar.activation(l1[:], ns[:],
                                         mybir.ActivationFunctionType.Ln, bias=1.0)
                    nc.scalar.activation(l2[:], ns[:],
                                         mybir.ActivationFunctionType.Ln, bias=c_e2[:])
                    # contrib = (lnE - lnEns + npad*l1 + l2 - 2) * scale
                    nc.vector.tensor_sub(u1[:], lnE[:], lnEns[:])
                    nc.vector.tensor_mul(u2[:], npad[:, b:b + 1], l1[:])
                    nc.vector.tensor_add(u1[:], u1[:], u2[:])
                    nc.vector.tensor_add(u1[:], u1[:], l2[:])
                    nc.vector.tensor_scalar(contrib[:, b:b + 1], u1[:], -2.0,
                                            scale[:, b:b + 1],
                                            mybir.AluOpType.add,
                                            mybir.AluOpType.mult)

            nc.sync.dma_start(out_d[:], contrib[:])

    nc.compile()
    return nc


def _in_maps(plan, features):
    features = np.ascontiguousarray(features, dtype=np.float32)
    maps = []
    for k in range(N_CORES):
        pc = plan["per_core"][k]
        maps.append(
            dict(
                feat=features[k].reshape(C, HW),
                gidx=pc["gidx"],
                widx=pc["widx"],
                npad=pc["npad"],
                scale=pc["scale"],
            )
        )
    return maps


def _loss_from_results(contribs):
    total = np.float32(0.0)
    for c in contribs:
        total += np.sum(c.astype(np.float32))
    return np.float32(-(total / np.float32(TV)))


class _Runner:
    """PJRT executor for the SPMD bass kernel, with device-resident inputs so
    repeated calls measure device execution (adapted from
    bass2jax.run_bass_via_pjrt)."""

    def __init__(self, nc, in_maps):
        import jax
        import mybir
        from jax.sharding import Mesh, PartitionSpec, NamedSharding
        from jax.experimental.shard_map import shard_map
        from concourse import bass2jax
        from concourse.bass2jax import _bass_exec_p, partition_id_tensor

        bass2jax.install_neuronx_cc_hook()
        n_cores = N_CORES
        partition_name = nc.partition_id_tensor.name if nc.partition_id_tensor else None

        in_names, out_names, out_avals, zero_outs = [], [], [], []
        for alloc in nc.m.functions[0].allocations:
            if not isinstance(alloc, mybir.MemoryLocationSet):
                continue
            name = alloc.memorylocations[0].name
            if alloc.kind == "ExternalInput":
                if name != partition_name:
                    in_names.append(name)
            elif alloc.kind == "ExternalOutput":
                dt_np = mybir.dt.np(alloc.dtype)
                out_names.append(name)
                out_avals.append(
                    jax.core.ShapedArray(tuple(alloc.tensor_shape), dt_np))
                zero_outs.append(np.zeros(alloc.tensor_shape, dt_np))
        if nc.dbg_addr is not None:
            in_maps = [{**m, nc.dbg_addr.name: np.zeros((1, 2), np.uint32)}
                       for m in in_maps]
        n_params = len(in_names)
        n_outs = len(out_avals)
        in_names = in_names + out_names
        if partition_name is not None:
            in_names.append(partition_name)

        def _body(*args):
            operands = list(args)
            if partition_name is not None:
                operands.append(partition_id_tensor())
            return tuple(_bass_exec_p.bind(
                *operands,
                out_avals=tuple(out_avals),
                in_names=tuple(in_names),
                out_names=tuple(out_names),
                lowering_input_output_aliases=(),
                sim_require_finite=True,
                sim_require_nnan=True,
                nc=nc,
            ))

        devices = jax.devices()[:n_cores]
        mesh = Mesh(np.asarray(devices), ("core",))
        donate = tuple(range(n_params, n_params + n_outs))
        self._fn = jax.jit(
            shard_map(_body, mesh=mesh,
                      in_specs=(PartitionSpec("core"),) * (n_params + n_outs),
                      out_specs=(PartitionSpec("core"),) * len(out_names)),
            donate_argnums=donate, keep_unused=True)

        sh = NamedSharding(mesh, PartitionSpec("core"))
        self._dev_in = [
            jax.device_put(
                np.concatenate([np.asarray(in_maps[c][nm]) for c in range(n_cores)],
                               axis=0), sh)
            for nm in in_names[:n_params]
        ]
        self._zero_outs = zero_outs
        self._sh = sh
        self._out_names = out_names
        self._out_avals = out_avals
        self._jax = jax

    def _zeros(self):
        import jax
        return [jax.device_put(
            np.zeros((N_CORES * z.shape[0], *z.shape[1:]), z.dtype), self._sh)
            for z in self._zero_outs]

    def run(self):
        out = self._fn(*self._dev_in, *self._zeros())
        return [
            {nm: np.asarray(out[i]).reshape(N_CORES, *self._out_avals[i].shape)[c]
             for i, nm in enumerate(self._out_names)}
            for c in range(N_CORES)
        ]

    def time_ns(self, iters=30):
        import time
        self.run()  # warm
        best = float("inf")
        for _ in range(iters):
            zs = self._zeros()
            for z in zs:
                z.block_until_ready()
            t0 = time.perf_counter_ns()
            out = self._fn(*self._dev_in, *zs)
            for o in out:
                o.block_until_ready()
            t1 = time.perf_counter_ns()
            best = min(best, t1 - t0)
        return best


def make_runner(features, batch_inds, sample_inds, labels):
    plan = _plan(batch_inds, sample_inds, labels)
    nc = _build(plan, debug=False)
    in_maps = _in_maps(plan, features)
    return _Runner(nc, in_maps)


def kernel(features, batch_inds, sample_inds, labels):
    r = make_runner(features, batch_inds, sample_inds, labels)
    results = r.run()
    contribs = [results[k]["contrib"] for k in range(N_CORES)]
    return _loss_from_results(contribs)


def kernel_sim(features, batch_inds, sample_inds, labels):
    """CoreSim-based check (no hardware)."""
    plan = _plan(batch_inds, sample_inds, labels)
    nc = _build(plan, debug=True)
    in_maps = _in_maps(plan, features)

    from concourse.bass_interp import MultiCoreSim

    sim = MultiCoreSim(nc, num_cores=N_CORES)
    for k, core in sim.cores.items():
        for name, arr in in_maps[k].items():
            core.tensor(name)[:] = arr
    sim.simulate(check_with_hw=False)
    contribs = [np.array(sim.cores[k].tensor("contrib")) for k in range(N_CORES)]
    return _loss_from_results(contribs)
